# revision 21
# baseline (speedup 1.0000x reference)
"""GCN (3-layer, PyG GCNConv-style) forward pass on 8 Trainium2 NeuronCores.

Strategy (graph/data parallel, edge-cut by destination):
  - Nodes are partitioned across 8 cores (dst ownership); edges live on the
    core that owns their destination. Weights are replicated.
  - Symmetric normalization deg^-1/2[src]*deg^-1/2[dst] is factored: tables
    store g = dinv * h, so aggregation is an unweighted gather-sum followed
    by a per-destination dinv scale.
  - Layer 1's slot array is a pure relayout of the (pre-scaled) input, so it
    is pre-gathered host-side and streamed with large contiguous DMAs.
  - Layers 2-3: dma_gather pulls g[src] rows (256B fp16) from the full table
    in local DRAM, with the four index buckets spread over the 4 SWDGE queues
    so descriptor generation runs on all four Q7 cpu pairs in parallel.
  - Onehot-selection matmuls (fp16) on the TensorEngine segment-sum the rows
    into per-dst-tile PSUM accumulators; a dense matmul applies W; the
    epilogue applies dinv/bias/relu; an AllGather rebuilds the full table for
    the next layer (the "halo exchange" -- on a random graph the halo is
    everything). Layer 3 ends in a fused log_softmax (fp32).
  - One-hots are built one chunk-column at a time with tensor_single_scalar
    (iota row == per-partition dstl scalar), keeping every DVE access pattern
    packed; pad slots carry dstl=-1 so they match no destination.
"""

import sys
from contextlib import ExitStack

import numpy as np

sys.path.insert(0, "/opt/trn_rl_repo")

import concourse.bacc as bacc  # noqa: E402
import concourse.tile as tile  # noqa: E402
from concourse import bass_utils, mybir  # noqa: E402

P = 128
NCORES = 8
NBUCK = 4
GROUP_TILES = 6
MAXGIDX = 4096  # SWDGE ring bound; >1024 needs single_packet=False
F32 = mybir.dt.float32
F16 = mybir.dt.float16
I16 = mybir.dt.int16


def _roundup(x, m):
    return (x + m - 1) // m * m


class _Plan:
    """Host-side preprocessing: edge partitioning, slot layout, input packing."""

    def __init__(self, x, edge_index, n_cores=NCORES):
        N, D = x.shape
        assert D == P, "feature dim must be 128"
        self.N, self.D = N, D
        nloc = -(-N // P) * P // n_cores if False else -(-N // n_cores)
        ptiles = -(-nloc // P)
        nlocp = ptiles * P
        npad = nlocp * n_cores
        assert npad % NBUCK == 0
        brows = npad // NBUCK
        assert brows <= 32767, "bucket rows must fit int16"
        assert nloc < nlocp, "need pad rows so pad-slot gathers stay in range"
        self.n_cores, self.nloc, self.ptiles = n_cores, nloc, ptiles
        self.nlocp, self.npad, self.brows = nlocp, npad, brows

        src = np.asarray(edge_index[0]).astype(np.int64)
        dst = np.asarray(edge_index[1]).astype(np.int64)
        # self-loops contribute +1 to every degree; they are applied on-device
        # via an identity matmul on the local slice, not via the gather
        deg = np.bincount(dst, minlength=N).astype(np.float64) + 1.0
        dinv = np.where(deg > 0, 1.0 / np.sqrt(deg), 0.0).astype(np.float32)
        self.dinv = dinv

        # padded global ids
        def gid(v):
            c = v // nloc
            return c * nlocp + (v - c * nloc)

        gsrc = gid(src)
        gdst = gid(dst)
        core = dst // nloc

        # per-core edge sets, counted per (tile, bucket)
        ecnt = np.zeros((n_cores, ptiles, NBUCK), dtype=np.int64)
        per_core = []
        for c in range(n_cores):
            m = core == c
            ld = gdst[m] - c * nlocp
            t = ld // P
            dl = (ld % P).astype(np.float32)
            gs = gsrc[m]
            b = gs // brows
            bi = (gs % brows).astype(np.int16)
            np.add.at(ecnt[c], (t, b), 1)
            per_core.append((t, b, bi, dl, gs))

        # exact per-(tile,bucket) capacities: max over cores, no alignment.
        # chunks may straddle tile boundaries; straddling chunks matmul into
        # both tiles' PSUM accumulators with windowed iota onehots.
        caps = np.zeros((ptiles, NBUCK), dtype=np.int64)
        for t in range(ptiles):
            for b in range(NBUCK):
                caps[t, b] = int(ecnt[:, t, b].max())
        self.caps = caps

        groups = []
        for g0 in range(0, ptiles, GROUP_TILES):
            groups.append(list(range(g0, min(g0 + GROUP_TILES, ptiles))))
        self.groups = groups

        # slot layout per group: bucket-major blocks (each padded to 128),
        # tile-order within bucket, exact caps between tiles
        self.g_nslots = []       # total slots per group
        self.g_gather = []       # per group: list over b of (num_idxs, col_start)
        self.g_bruns = []        # per group: {t: [(c0, c1) col ranges per bucket]}
        for tiles_g in groups:
            slot = 0
            gathers = []
            bruns = {t: [] for t in tiles_g}
            for b in range(NBUCK):
                bstart = slot
                for t in tiles_g:
                    cap = int(caps[t, b])
                    if cap:
                        c0 = (slot - bstart) // P + bstart // P
                        c1 = -(-(slot + cap - bstart) // P) + bstart // P
                        bruns[t].append((c0, c1))
                    slot += cap
                nb = _roundup(slot - bstart, P)
                slot = bstart + nb
                gathers.append((nb, bstart // P))
            self.g_nslots.append(slot)
            self.g_gather.append(gathers)
            self.g_bruns.append(bruns)
        # merge each tile's per-bucket col ranges into a sorted unique col list
        self.g_chunkcols = []
        for gi, tiles_g in enumerate(groups):
            chunkcols = {}
            for t in tiles_g:
                cols = []
                for (c0, c1) in self.g_bruns[gi][t]:
                    cols.extend(range(c0, c1))
                chunkcols[t] = sorted(set(cols))
            self.g_chunkcols.append(chunkcols)
        self.nch_groups = [ns // P for ns in self.g_nslots]
        self.cch = int(sum(self.nch_groups))
        self.c16 = int(sum(ns // 16 for ns in self.g_nslots))
        self.nmax = max(
            (c1 - c0)
            for gi in range(len(groups))
            for t in groups[gi]
            for (c0, c1) in self.g_bruns[gi][t])
        assert self.nmax <= 16

        # Per (group, bucket): number of leading slots every core must gather
        # (max over cores of last-real-slot+1, bucket-block-relative). Slots
        # beyond it get idx=-1, which the Q7 descriptor generator skips; the
        # shared num_idxs_reg must equal the per-core non-negative idx count,
        # so the cutoff is uniform across cores.
        self.g_vreg = []
        for gi, tiles_g in enumerate(groups):
            vreg = []
            s = 0
            for b in range(NBUCK):
                bstart = s
                last = 0
                for t in tiles_g:
                    cap = int(caps[t, b])
                    if cap:
                        cnt_max = int(ecnt[:, t, b].max())
                        last = (s - bstart) + cnt_max
                    s += cap
                nb = _roundup(s - bstart, P)
                s = bstart + nb
                vreg.append(last)
            assert s == self.g_nslots[gi]
            vreg = [min(v, nb_ci[0]) for v, nb_ci in
                    zip(vreg, self.g_gather[gi])]
            self.g_vreg.append(vreg)

        # pack per-core gidx / dstl / slot gids (for the layer-1 pre-gather).
        # Pad slots gather row 0 of their bucket (real, finite data) and carry
        # dstl=-1, so the onehot weights their contribution to zero.
        self.per_core_inputs = []
        self.per_core_gids = []
        for c in range(n_cores):
            t_, b_, bi_, dl_, gs_ = per_core[c]
            order = np.lexsort((b_, t_))
            t_, b_, bi_, dl_, gs_ = (t_[order], b_[order], bi_[order],
                                     dl_[order], gs_[order])
            # bucket arrays grouped by (t) within bucket
            gidx = np.zeros((128, self.c16), dtype=np.int16)
            dstl = np.zeros((128, self.cch), dtype=np.float32)
            gid_all = []
            off16 = 0
            chbase = 0
            for gi, tiles_g in enumerate(groups):
                g0 = tiles_g[0]
                ns_g = self.g_nslots[gi]
                slots_idx = np.zeros(ns_g, dtype=np.int16)
                slots_dl = np.full(ns_g, -1.0, dtype=np.float32)
                slots_gid = np.full(ns_g, -1, dtype=np.int64)
                s = 0
                for b in range(NBUCK):
                    bstart = s
                    for t in tiles_g:
                        cap = int(caps[t, b])
                        if cap == 0:
                            continue
                        m = (t_ == t) & (b_ == b)
                        cnt = int(m.sum())
                        slots_idx[s:s + cnt] = bi_[m]
                        # dstl is group-relative (0..GROUP_TILES*128)
                        slots_dl[s:s + cnt] = (t - g0) * P + dl_[m]
                        slots_gid[s:s + cnt] = gs_[m]
                        s += cap
                    nb = _roundup(s - bstart, P)
                    s = bstart + nb
                assert s == ns_g
                # wrapped-16 index layout, replicated to 128 partitions
                blk = slots_idx.reshape(-1, 16).T  # [16, ns/16]
                gidx[:, off16:off16 + ns_g // 16] = np.tile(blk, (8, 1))
                # dstl: slot s -> [s%128, chbase + s//128]
                dstl[:, chbase:chbase + ns_g // P] = (
                    slots_dl.reshape(-1, 128).T)
                gid_all.append(slots_gid)
                off16 += ns_g // 16
                chbase += ns_g // P
            dexp = np.repeat(dstl.astype(np.float16)[:, :, None], P, axis=2)
            self.per_core_inputs.append((gidx, dstl, dexp))
            self.per_core_gids.append(gid_all)

        # per-core xpre (dinv*x, padded, fp16) and packed dinv
        self.xloc = []
        self.dinvl = []
        self.dinv2l = []
        self.invdl = []
        xpre32 = np.zeros((npad, D), dtype=np.float32)
        for c in range(n_cores):
            lo = c * nloc
            hi = min(lo + nloc, N)
            xpre32[c * nlocp:c * nlocp + hi - lo] = (
                np.asarray(x[lo:hi]) * dinv[lo:hi, None])
            self.xloc.append(
                xpre32[c * nlocp:(c + 1) * nlocp].astype(np.float16))
            dv = np.zeros(nlocp, dtype=np.float32)
            dv[: hi - lo] = dinv[lo:hi]
            self.dinvl.append(dv.reshape(ptiles, P).T.copy())  # [128, ptiles]
            self.dinv2l.append((dv * dv).reshape(ptiles, P).T.copy())
            iv = np.zeros(nlocp, dtype=np.float32)
            nz = dv > 0
            iv[nz] = 1.0 / dv[nz]
            self.invdl.append(iv.astype(np.float16)[None, :].copy())  # [1, nlocp]
        xpre16 = xpre32.astype(np.float16)
        # layer-1 slot array pre-gathered host-side: [128, cch, 128] fp16
        self.gt1 = []
        for c in range(n_cores):
            parts = []
            for gi in range(len(groups)):
                sg = self.per_core_gids[c][gi]
                vals = np.where(sg[:, None] >= 0, xpre16[np.maximum(sg, 0)],
                                np.float16(0.0))
                nch_g = self.g_nslots[gi] // P
                parts.append(vals.reshape(nch_g, P, P).transpose(1, 0, 2))
            self.gt1.append(np.ascontiguousarray(
                np.concatenate(parts, axis=1), dtype=np.float16))


def _build_program(plan, dout, has_bias=True):
    n_cores = plan.n_cores
    nlocp, npad, brows, ptiles = plan.nlocp, plan.npad, plan.brows, plan.ptiles
    nc = bacc.Bacc("TRN2", target_bir_lowering=False, debug=False,
                   num_devices=n_cores, num_swdge_queues=4)

    xloc = nc.dram_tensor("xloc", [nlocp, P], F16, kind="ExternalInput")
    gt1d = nc.dram_tensor("gt1", [128, plan.cch, P], F16, kind="ExternalInput")
    gidx = nc.dram_tensor("gidx", [128, plan.c16], I16, kind="ExternalInput")
    dexpd = nc.dram_tensor("dexp", [128, plan.cch, P], F16,
                           kind="ExternalInput")
    dinvl = nc.dram_tensor("dinvl", [128, ptiles], F32, kind="ExternalInput")
    dinv2 = nc.dram_tensor("dinv2", [128, ptiles], F32, kind="ExternalInput")
    invdd = nc.dram_tensor("invd", [1, nlocp], F16, kind="ExternalInput")
    w1 = nc.dram_tensor("w1", [P, P], F16, kind="ExternalInput")
    w2 = nc.dram_tensor("w2", [P, P], F16, kind="ExternalInput")
    w3 = nc.dram_tensor("w3", [P, dout], F16, kind="ExternalInput")
    b1r = nc.dram_tensor("b1r", [1, P], F16, kind="ExternalInput")
    b2r = nc.dram_tensor("b2r", [1, P], F16, kind="ExternalInput")
    b3r = nc.dram_tensor("b3r", [1, dout], F16, kind="ExternalInput")
    iotr = nc.dram_tensor("iotr", [P, GROUP_TILES, plan.nmax, P], F16,
                          kind="ExternalInput")
    ident = nc.dram_tensor("ident", [P, P], F16, kind="ExternalInput")
    out = nc.dram_tensor("out", [nlocp, dout], F32, kind="ExternalOutput")

    rg = [list(range(n_cores))]
    nchmax = max(plan.nch_groups)

    with tile.TileContext(nc) as tc, ExitStack() as ctx:
        # ---- constants in SBUF
        cpool = ctx.enter_context(tc.tile_pool(name="const", bufs=1))
        w1s = cpool.tile([P, P], F16)
        w2s = cpool.tile([P, P], F16)
        w3s = cpool.tile([P, dout], F16)
        b1s = cpool.tile([1, P], F16)
        b2s = cpool.tile([1, P], F16)
        b3s = cpool.tile([1, dout], F16)
        iot = cpool.tile([P, GROUP_TILES, plan.nmax, P], F16)
        ids = cpool.tile([P, P], F16)
        dvs = cpool.tile([P, ptiles], F32)
        dv2s = cpool.tile([P, ptiles], F32)
        ivs = cpool.tile([1, nlocp], F16)
        its = cpool.tile([128, plan.c16], I16)
        for t_, d_ in ((w1s, w1), (w2s, w2), (w3s, w3), (b1s, b1r),
                       (b2s, b2r), (b3s, b3r), (iot, iotr), (ids, ident),
                       (dvs, dinvl), (dv2s, dinv2), (ivs, invdd),
                       (its, gidx)):
            nc.sync.dma_start(out=t_[:], in_=d_[:])

        # ---- DRAM intermediates
        dram = ctx.enter_context(tc.tile_pool(name="dram", bufs=1, space="DRAM"))
        g1loc = dram.tile([nlocp, P], F16)
        g1full = dram.tile([npad, P], F16, addr_space="Shared")
        g2loc = dram.tile([nlocp, P], F16)
        g2full = dram.tile([npad, P], F16, addr_space="Shared")

        gpool = ctx.enter_context(tc.tile_pool(name="gbuf", bufs=2))
        depool = ctx.enter_context(tc.tile_pool(name="dexp", bufs=2))
        ohpool = ctx.enter_context(tc.tile_pool(name="oh", bufs=5))
        stpool = ctx.enter_context(tc.tile_pool(name="st", bufs=4))
        gspool = ctx.enter_context(tc.tile_pool(name="gs", bufs=4))
        hpool = ctx.enter_context(tc.tile_pool(name="hs", bufs=6))
        pspool = ctx.enter_context(tc.tile_pool(name="ps", bufs=2, space="PSUM"))
        ptpool = ctx.enter_context(tc.tile_pool(name="pt", bufs=4, space="PSUM"))

        layers = (
            (None, xloc, g1loc, w1s, b1s, P, True),
            (g1full, g1loc, g2loc, w2s, b2s, P, True),
            (g2full, g2loc, None, w3s, b3s, dout, False),
        )
        for li, (table, ltable, gout, ws, bs, do_, isrelu) in enumerate(layers):
            off16 = 0
            chbase = 0
            for gi, tiles_g in enumerate(plan.groups):
                nch_g = plan.nch_groups[gi]
                gt = gpool.tile([P, nchmax, P], F16, tag="G")
                if table is None:
                    # layer 1: slot array was pre-gathered host-side
                    nc.sync.dma_start(out=gt[:, :nch_g, :],
                                      in_=gt1d[:, chbase:chbase + nch_g, :])
                    for b in range(NBUCK):
                        nb, _ = plan.g_gather[gi][b]
                        off16 += nb // 16
                else:
                    for b in range(NBUCK):
                        nb, cstart = plan.g_gather[gi][b]
                        if nb == 0:
                            continue
                        n16 = nb // 16
                        for s0 in range(0, nb, MAXGIDX):
                            m = min(MAXGIDX, nb - s0)
                            nc.gpsimd.dma_gather(
                                gt[:, cstart + s0 // P:cstart + (s0 + m) // P, :],
                                table[b * brows:(b + 1) * brows, :],
                                its[:, off16 + s0 // 16:off16 + (s0 + m) // 16],
                                m, m, P, single_packet=False, queue_num=b)
                        off16 += n16
                de_ = depool.tile([P, nchmax, P], F16, tag="DE")
                nc.sync.dma_start(out=de_[:, :nch_g, :],
                                  in_=dexpd[:, chbase:chbase + nch_g, :])
                chbase += nch_g

                l3state = []
                for t in tiles_g:
                    tl = t - tiles_g[0]
                    runs = plan.g_bruns[gi][t]
                    cols = [c for (c0, c1) in runs for c in range(c0, c1)]
                    gself = gspool.tile([P, P], F16, tag="gs")
                    nc.sync.dma_start(out=gself[:],
                                      in_=ltable[t * P:(t + 1) * P, :])
                    if cols:
                        ncht = len(cols)
                        oh = ohpool.tile([P, ncht, P], F16, tag="oh")
                        l0 = 0
                        for (c0, c1) in runs:
                            n = c1 - c0
                            nc.vector.tensor_tensor(
                                out=oh[:, l0:l0 + n, :],
                                in0=de_[:, c0:c1, :],
                                in1=iot[:, tl, 0:n, :],
                                op=mybir.AluOpType.is_equal)
                            l0 += n
                    ps = pspool.tile([P, P], F32, tag="ps", space="PSUM")
                    for j, col in enumerate(cols):
                        nc.tensor.matmul(
                            ps[:], lhsT=gt[:, col, :], rhs=oh[:, j, :],
                            start=(j == 0), stop=False)
                    # self-loop: S^T += gself^T (identity matmul, local rows)
                    nc.tensor.matmul(ps[:], lhsT=gself[:], rhs=ids[:],
                                     start=(len(cols) == 0), stop=True)
                    st = stpool.tile([P, P], F16, tag="st")
                    nc.scalar.copy(out=st[:], in_=ps[:])
                    pt = ptpool.tile([P, do_], F32, tag="pt", space="PSUM")
                    if has_bias:
                        # bias via rank-1 update: pt += (1/dv)[dst] x b[f], so
                        # the dv (or dv^2) scale below also adds the +b term
                        nc.tensor.matmul(pt[:], lhsT=st[:], rhs=ws[:, :do_],
                                         start=True, stop=False)
                        nc.tensor.matmul(pt[:], lhsT=ivs[:, t * P:(t + 1) * P],
                                         rhs=bs[:, :do_], start=False,
                                         stop=True)
                    else:
                        nc.tensor.matmul(pt[:], lhsT=st[:], rhs=ws[:, :do_],
                                         start=True, stop=True)
                    dv = dvs[:, t:t + 1]
                    if isrelu:
                        # dv*relu(dv*raw + b) == relu(dv^2*(raw + b/dv));
                        # fused mult+max on DVE keeps the Act queue free for
                        # the st copies that gate the W matmuls
                        h2 = hpool.tile([P, do_], F16, tag="h2")
                        nc.vector.tensor_scalar(
                            out=h2[:], in0=pt[:],
                            scalar1=dv2s[:, t:t + 1], scalar2=0.0,
                            op0=mybir.AluOpType.mult,
                            op1=mybir.AluOpType.max)
                        nc.sync.dma_start(out=gout[t * P:(t + 1) * P, :],
                                          in_=h2[:])
                    else:
                        # log_softmax, batched over the group's tiles so the
                        # Exp/Ln activation tables each load once per group
                        # (not per tile) and the final DVE subtract never
                        # waits at the queue head
                        hs = hpool.tile([P, do_], F32, tag="hs")
                        nc.vector.tensor_scalar_mul(out=hs[:], in0=pt[:],
                                                    scalar1=dv)
                        l3state.append((t, dv, pt, hs))
                if l3state:
                    # no max-subtraction: |h3| stays far below fp32 exp range
                    sms = []
                    for t, dv, pt, hs in l3state:
                        es = hpool.tile([P, do_], F32, tag="es")
                        sm = hpool.tile([P, 1], F32, tag="sm")
                        nc.scalar.activation(
                            out=es[:], in_=pt[:],
                            func=mybir.ActivationFunctionType.Exp,
                            scale=dv, accum_out=sm[:])
                        sms.append(sm)
                    lss = []
                    for sm in sms:
                        ls = hpool.tile([P, 1], F32, tag="ls")
                        nc.scalar.activation(
                            out=ls[:], in_=sm[:],
                            func=mybir.ActivationFunctionType.Ln)
                        lss.append(ls)
                    for (t, dv, pt, hs), ls in zip(l3state, lss):
                        os_ = hpool.tile([P, do_], F32, tag="os")
                        nc.vector.tensor_single_scalar(
                            out=os_[:], in_=hs[:], scalar=ls[:],
                            op=mybir.AluOpType.subtract)
                        nc.sync.dma_start(out=out[t * P:(t + 1) * P, :],
                                          in_=os_[:])
            if li == 0:
                nc.gpsimd.collective_compute(
                    "AllGather", mybir.AluOpType.bypass, replica_groups=rg,
                    ins=[g1loc[:, :]], outs=[g1full[:, :]])
            elif li == 1:
                nc.gpsimd.collective_compute(
                    "AllGather", mybir.AluOpType.bypass, replica_groups=rg,
                    ins=[g2loc[:, :]], outs=[g2full[:, :]])

    nc.compile()
    return nc


def _make_in_maps(plan, W1, b1, W2, b2, W3, b3):
    dout = W3.shape[1]
    # iotr[p, tl, j, d] = tl*128 + d  (repeated iota windows per group tile)
    base = np.arange(P, dtype=np.float32)
    iotr = np.zeros((P, GROUP_TILES, plan.nmax, P), dtype=np.float16)
    for tl in range(GROUP_TILES):
        iotr[:, tl, :, :] = (tl * P + base)[None, None, :]
    common = {
        "w1": np.asarray(W1, np.float16), "w2": np.asarray(W2, np.float16),
        "w3": np.asarray(W3, np.float16),
        "b1r": np.asarray(b1, np.float16)[None, :],
        "b2r": np.asarray(b2, np.float16)[None, :],
        "b3r": np.asarray(b3, np.float16)[None, :],
        "iotr": iotr,
        "ident": np.eye(P, dtype=np.float16),
    }
    in_maps = []
    for c in range(plan.n_cores):
        gidx, dstl, dexp = plan.per_core_inputs[c]
        m = dict(common)
        m["xloc"] = plan.xloc[c]
        m["gt1"] = plan.gt1[c]
        m["gidx"] = gidx
        m["dexp"] = dexp
        m["dinvl"] = plan.dinvl[c]
        m["dinv2"] = plan.dinv2l[c]
        m["invd"] = plan.invdl[c]
        in_maps.append(m)
    return in_maps


def run_gcn(x, edge_index, W1, b1, W2, b2, W3, b3, n_cores=NCORES,
            trace=False):
    plan = _Plan(np.asarray(x, np.float32), edge_index, n_cores)
    dout = np.asarray(W3).shape[1]
    has_bias = any(np.any(np.asarray(b)) for b in (b1, b2, b3))
    nc = _build_program(plan, dout, has_bias)
    in_maps = _make_in_maps(plan, W1, b1, W2, b2, W3, b3)
    res = bass_utils.run_bass_kernel_spmd(
        nc, in_maps, core_ids=list(range(n_cores)), trace=trace)
    outs = []
    for c in range(n_cores):
        lo = c * plan.nloc
        hi = min(lo + plan.nloc, plan.N)
        outs.append(res.results[c]["out"][: hi - lo])
    full = np.concatenate(outs, axis=0)
    return full, res


def kernel(x, edge_index, W1, b1, W2, b2, W3, b3):
    out, _ = run_gcn(x, edge_index, W1, b1, W2, b2, W3, b3)
    return out


# revision 22
# speedup vs baseline: 1.0059x; 1.0059x over previous
"""GCN (3-layer, PyG GCNConv-style) forward pass on 8 Trainium2 NeuronCores.

Strategy (graph/data parallel, edge-cut by destination):
  - Nodes are partitioned across 8 cores (dst ownership); edges live on the
    core that owns their destination. Weights are replicated.
  - Symmetric normalization deg^-1/2[src]*deg^-1/2[dst] is factored: tables
    store g = dinv * h, so aggregation is an unweighted gather-sum followed
    by a per-destination dinv scale.
  - Layer 1's slot array is a pure relayout of the (pre-scaled) input, so it
    is pre-gathered host-side and streamed with large contiguous DMAs.
  - Layers 2-3: dma_gather pulls g[src] rows (256B fp16) from the full table
    in local DRAM, with the four index buckets spread over the 4 SWDGE queues
    so descriptor generation runs on all four Q7 cpu pairs in parallel.
  - Onehot-selection matmuls (fp16) on the TensorEngine segment-sum the rows
    into per-dst-tile PSUM accumulators; a dense matmul applies W; the
    epilogue applies dinv/bias/relu; an AllGather rebuilds the full table for
    the next layer (the "halo exchange" -- on a random graph the halo is
    everything). Layer 3 ends in a fused log_softmax (fp32).
  - One-hots are built one chunk-column at a time with tensor_single_scalar
    (iota row == per-partition dstl scalar), keeping every DVE access pattern
    packed; pad slots carry dstl=-1 so they match no destination.
"""

import sys
from contextlib import ExitStack

import numpy as np

sys.path.insert(0, "/opt/trn_rl_repo")

import concourse.bacc as bacc  # noqa: E402
import concourse.tile as tile  # noqa: E402
from concourse import bass_utils, mybir  # noqa: E402

P = 128
NCORES = 8
NBUCK = 4
GROUP_TILES = 6
MAXGIDX = 4096  # SWDGE ring bound; >1024 needs single_packet=False
F32 = mybir.dt.float32
F16 = mybir.dt.float16
I16 = mybir.dt.int16


def _roundup(x, m):
    return (x + m - 1) // m * m


class _Plan:
    """Host-side preprocessing: edge partitioning, slot layout, input packing."""

    def __init__(self, x, edge_index, n_cores=NCORES):
        N, D = x.shape
        assert D == P, "feature dim must be 128"
        self.N, self.D = N, D
        nloc = -(-N // P) * P // n_cores if False else -(-N // n_cores)
        ptiles = -(-nloc // P)
        nlocp = ptiles * P
        npad = nlocp * n_cores
        assert npad % NBUCK == 0
        brows = npad // NBUCK
        assert brows <= 32767, "bucket rows must fit int16"
        assert nloc < nlocp, "need pad rows so pad-slot gathers stay in range"
        self.n_cores, self.nloc, self.ptiles = n_cores, nloc, ptiles
        self.nlocp, self.npad, self.brows = nlocp, npad, brows

        src = np.asarray(edge_index[0]).astype(np.int64)
        dst = np.asarray(edge_index[1]).astype(np.int64)
        # self-loops contribute +1 to every degree; they are applied on-device
        # via an identity matmul on the local slice, not via the gather
        deg = np.bincount(dst, minlength=N).astype(np.float64) + 1.0
        dinv = np.where(deg > 0, 1.0 / np.sqrt(deg), 0.0).astype(np.float32)
        self.dinv = dinv

        # padded global ids
        def gid(v):
            c = v // nloc
            return c * nlocp + (v - c * nloc)

        gsrc = gid(src)
        gdst = gid(dst)
        core = dst // nloc

        # per-core edge sets, counted per (tile, bucket)
        ecnt = np.zeros((n_cores, ptiles, NBUCK), dtype=np.int64)
        per_core = []
        for c in range(n_cores):
            m = core == c
            ld = gdst[m] - c * nlocp
            t = ld // P
            dl = (ld % P).astype(np.float32)
            gs = gsrc[m]
            b = gs // brows
            bi = (gs % brows).astype(np.int16)
            np.add.at(ecnt[c], (t, b), 1)
            per_core.append((t, b, bi, dl, gs))

        # exact per-(tile,bucket) capacities: max over cores, no alignment.
        # chunks may straddle tile boundaries; straddling chunks matmul into
        # both tiles' PSUM accumulators with windowed iota onehots.
        caps = np.zeros((ptiles, NBUCK), dtype=np.int64)
        for t in range(ptiles):
            for b in range(NBUCK):
                caps[t, b] = int(ecnt[:, t, b].max())
        self.caps = caps

        groups = []
        for g0 in range(0, ptiles, GROUP_TILES):
            groups.append(list(range(g0, min(g0 + GROUP_TILES, ptiles))))
        self.groups = groups

        # slot layout per group: bucket-major blocks (each padded to 128),
        # tile-order within bucket, exact caps between tiles
        self.g_nslots = []       # total slots per group
        self.g_gather = []       # per group: list over b of (num_idxs, col_start)
        self.g_bruns = []        # per group: {t: [(c0, c1) col ranges per bucket]}
        for tiles_g in groups:
            slot = 0
            gathers = []
            bruns = {t: [] for t in tiles_g}
            for b in range(NBUCK):
                bstart = slot
                for t in tiles_g:
                    cap = int(caps[t, b])
                    if cap:
                        c0 = (slot - bstart) // P + bstart // P
                        c1 = -(-(slot + cap - bstart) // P) + bstart // P
                        bruns[t].append((c0, c1))
                    slot += cap
                nb = _roundup(slot - bstart, P)
                slot = bstart + nb
                gathers.append((nb, bstart // P))
            self.g_nslots.append(slot)
            self.g_gather.append(gathers)
            self.g_bruns.append(bruns)
        # merge each tile's per-bucket col ranges into a sorted unique col list
        self.g_chunkcols = []
        for gi, tiles_g in enumerate(groups):
            chunkcols = {}
            for t in tiles_g:
                cols = []
                for (c0, c1) in self.g_bruns[gi][t]:
                    cols.extend(range(c0, c1))
                chunkcols[t] = sorted(set(cols))
            self.g_chunkcols.append(chunkcols)
        self.nch_groups = [ns // P for ns in self.g_nslots]
        self.cch = int(sum(self.nch_groups))
        self.c16 = int(sum(ns // 16 for ns in self.g_nslots))
        self.nmax = max(
            (c1 - c0)
            for gi in range(len(groups))
            for t in groups[gi]
            for (c0, c1) in self.g_bruns[gi][t])
        assert self.nmax <= 16

        # Per (group, bucket): number of leading slots every core must gather
        # (max over cores of last-real-slot+1, bucket-block-relative). Slots
        # beyond it get idx=-1, which the Q7 descriptor generator skips; the
        # shared num_idxs_reg must equal the per-core non-negative idx count,
        # so the cutoff is uniform across cores.
        self.g_vreg = []
        for gi, tiles_g in enumerate(groups):
            vreg = []
            s = 0
            for b in range(NBUCK):
                bstart = s
                last = 0
                for t in tiles_g:
                    cap = int(caps[t, b])
                    if cap:
                        cnt_max = int(ecnt[:, t, b].max())
                        last = (s - bstart) + cnt_max
                    s += cap
                nb = _roundup(s - bstart, P)
                s = bstart + nb
                vreg.append(last)
            assert s == self.g_nslots[gi]
            vreg = [min(v, nb_ci[0]) for v, nb_ci in
                    zip(vreg, self.g_gather[gi])]
            self.g_vreg.append(vreg)

        # pack per-core gidx / dstl / slot gids (for the layer-1 pre-gather).
        # Pad slots gather row 0 of their bucket (real, finite data) and carry
        # dstl=-1, so the onehot weights their contribution to zero.
        self.per_core_inputs = []
        self.per_core_gids = []
        for c in range(n_cores):
            t_, b_, bi_, dl_, gs_ = per_core[c]
            order = np.lexsort((b_, t_))
            t_, b_, bi_, dl_, gs_ = (t_[order], b_[order], bi_[order],
                                     dl_[order], gs_[order])
            # bucket arrays grouped by (t) within bucket
            gidx = np.zeros((128, self.c16), dtype=np.int16)
            dstl = np.zeros((128, self.cch), dtype=np.float32)
            gid_all = []
            off16 = 0
            chbase = 0
            for gi, tiles_g in enumerate(groups):
                g0 = tiles_g[0]
                ns_g = self.g_nslots[gi]
                slots_idx = np.zeros(ns_g, dtype=np.int16)
                slots_dl = np.full(ns_g, -1.0, dtype=np.float32)
                slots_gid = np.full(ns_g, -1, dtype=np.int64)
                s = 0
                for b in range(NBUCK):
                    bstart = s
                    for t in tiles_g:
                        cap = int(caps[t, b])
                        if cap == 0:
                            continue
                        m = (t_ == t) & (b_ == b)
                        cnt = int(m.sum())
                        slots_idx[s:s + cnt] = bi_[m]
                        # dstl is group-relative (0..GROUP_TILES*128)
                        slots_dl[s:s + cnt] = (t - g0) * P + dl_[m]
                        slots_gid[s:s + cnt] = gs_[m]
                        s += cap
                    nb = _roundup(s - bstart, P)
                    s = bstart + nb
                assert s == ns_g
                # wrapped-16 index layout, replicated to 128 partitions
                blk = slots_idx.reshape(-1, 16).T  # [16, ns/16]
                gidx[:, off16:off16 + ns_g // 16] = np.tile(blk, (8, 1))
                # dstl: slot s -> [s%128, chbase + s//128]
                dstl[:, chbase:chbase + ns_g // P] = (
                    slots_dl.reshape(-1, 128).T)
                gid_all.append(slots_gid)
                off16 += ns_g // 16
                chbase += ns_g // P
            dexp = np.repeat(dstl.astype(np.float16)[:, :, None], P, axis=2)
            self.per_core_inputs.append((gidx, dstl, dexp))
            self.per_core_gids.append(gid_all)

        # per-core xpre (dinv*x, padded, fp16) and packed dinv
        self.xloc = []
        self.dinvl = []
        self.dinv2l = []
        self.invdl = []
        xpre32 = np.zeros((npad, D), dtype=np.float32)
        for c in range(n_cores):
            lo = c * nloc
            hi = min(lo + nloc, N)
            xpre32[c * nlocp:c * nlocp + hi - lo] = (
                np.asarray(x[lo:hi]) * dinv[lo:hi, None])
            self.xloc.append(
                xpre32[c * nlocp:(c + 1) * nlocp].astype(np.float16))
            dv = np.zeros(nlocp, dtype=np.float32)
            dv[: hi - lo] = dinv[lo:hi]
            self.dinvl.append(dv.reshape(ptiles, P).T.copy())  # [128, ptiles]
            self.dinv2l.append((dv * dv).reshape(ptiles, P).T.copy())
            iv = np.zeros(nlocp, dtype=np.float32)
            nz = dv > 0
            iv[nz] = 1.0 / dv[nz]
            self.invdl.append(iv.astype(np.float16)[None, :].copy())  # [1, nlocp]
        xpre16 = xpre32.astype(np.float16)
        # layer-1 slot array pre-gathered host-side: [128, cch, 128] fp16
        self.gt1 = []
        for c in range(n_cores):
            parts = []
            for gi in range(len(groups)):
                sg = self.per_core_gids[c][gi]
                vals = np.where(sg[:, None] >= 0, xpre16[np.maximum(sg, 0)],
                                np.float16(0.0))
                nch_g = self.g_nslots[gi] // P
                parts.append(vals.reshape(nch_g, P, P).transpose(1, 0, 2))
            self.gt1.append(np.ascontiguousarray(
                np.concatenate(parts, axis=1), dtype=np.float16))


def _build_program(plan, dout, has_bias=True):
    n_cores = plan.n_cores
    nlocp, npad, brows, ptiles = plan.nlocp, plan.npad, plan.brows, plan.ptiles
    nc = bacc.Bacc("TRN2", target_bir_lowering=False, debug=False,
                   num_devices=n_cores, num_swdge_queues=4)

    xloc = nc.dram_tensor("xloc", [nlocp, P], F16, kind="ExternalInput")
    gt1d = nc.dram_tensor("gt1", [128, plan.cch, P], F16, kind="ExternalInput")
    gidx = nc.dram_tensor("gidx", [128, plan.c16], I16, kind="ExternalInput")
    dexpd = nc.dram_tensor("dexp", [128, plan.cch, P], F16,
                           kind="ExternalInput")
    dinvl = nc.dram_tensor("dinvl", [128, ptiles], F32, kind="ExternalInput")
    dinv2 = nc.dram_tensor("dinv2", [128, ptiles], F32, kind="ExternalInput")
    invdd = nc.dram_tensor("invd", [1, nlocp], F16, kind="ExternalInput")
    w1 = nc.dram_tensor("w1", [P, P], F16, kind="ExternalInput")
    w2 = nc.dram_tensor("w2", [P, P], F16, kind="ExternalInput")
    w3 = nc.dram_tensor("w3", [P, dout], F16, kind="ExternalInput")
    b1r = nc.dram_tensor("b1r", [1, P], F16, kind="ExternalInput")
    b2r = nc.dram_tensor("b2r", [1, P], F16, kind="ExternalInput")
    b3r = nc.dram_tensor("b3r", [1, dout], F16, kind="ExternalInput")
    iotr = nc.dram_tensor("iotr", [P, GROUP_TILES, plan.nmax, P], F16,
                          kind="ExternalInput")
    ident = nc.dram_tensor("ident", [P, P], F16, kind="ExternalInput")
    out = nc.dram_tensor("out", [nlocp, dout], F32, kind="ExternalOutput")

    rg = [list(range(n_cores))]
    nchmax = max(plan.nch_groups)

    with tile.TileContext(nc) as tc, ExitStack() as ctx:
        # ---- constants in SBUF
        cpool = ctx.enter_context(tc.tile_pool(name="const", bufs=1))
        w1s = cpool.tile([P, P], F16)
        w2s = cpool.tile([P, P], F16)
        w3s = cpool.tile([P, dout], F16)
        b1s = cpool.tile([1, P], F16)
        b2s = cpool.tile([1, P], F16)
        b3s = cpool.tile([1, dout], F16)
        iot = cpool.tile([P, GROUP_TILES, plan.nmax, P], F16)
        ids = cpool.tile([P, P], F16)
        dvs = cpool.tile([P, ptiles], F32)
        dv2s = cpool.tile([P, ptiles], F32)
        ivs = cpool.tile([1, nlocp], F16)
        its = cpool.tile([128, plan.c16], I16)
        for t_, d_ in ((w1s, w1), (w2s, w2), (w3s, w3), (b1s, b1r),
                       (b2s, b2r), (b3s, b3r), (iot, iotr), (ids, ident),
                       (dvs, dinvl), (dv2s, dinv2), (ivs, invdd),
                       (its, gidx)):
            nc.sync.dma_start(out=t_[:], in_=d_[:])

        # ---- DRAM intermediates
        dram = ctx.enter_context(tc.tile_pool(name="dram", bufs=1, space="DRAM"))
        g1loc = dram.tile([nlocp, P], F16)
        g1full = dram.tile([npad, P], F16, addr_space="Shared")
        g2loc = dram.tile([nlocp, P], F16)
        g2full = dram.tile([npad, P], F16, addr_space="Shared")

        gpool = ctx.enter_context(tc.tile_pool(name="gbuf", bufs=2))
        depool = ctx.enter_context(tc.tile_pool(name="dexp", bufs=2))
        ohpool = ctx.enter_context(tc.tile_pool(name="oh", bufs=4))
        stpool = ctx.enter_context(tc.tile_pool(name="st", bufs=4))
        gspool = ctx.enter_context(tc.tile_pool(name="gs", bufs=4))
        hpool = ctx.enter_context(tc.tile_pool(name="hs", bufs=6))
        pspool = ctx.enter_context(tc.tile_pool(name="ps", bufs=2, space="PSUM"))
        ptpool = ctx.enter_context(tc.tile_pool(name="pt", bufs=2, space="PSUM"))

        layers = (
            (None, xloc, g1loc, w1s, b1s, P, True),
            (g1full, g1loc, g2loc, w2s, b2s, P, True),
            (g2full, g2loc, None, w3s, b3s, dout, False),
        )
        for li, (table, ltable, gout, ws, bs, do_, isrelu) in enumerate(layers):
            off16 = 0
            chbase = 0
            for gi, tiles_g in enumerate(plan.groups):
                nch_g = plan.nch_groups[gi]
                gt = gpool.tile([P, nchmax, P], F16, tag="G")
                if table is None:
                    # layer 1: slot array was pre-gathered host-side
                    nc.sync.dma_start(out=gt[:, :nch_g, :],
                                      in_=gt1d[:, chbase:chbase + nch_g, :])
                    for b in range(NBUCK):
                        nb, _ = plan.g_gather[gi][b]
                        off16 += nb // 16
                else:
                    for b in range(NBUCK):
                        nb, cstart = plan.g_gather[gi][b]
                        if nb == 0:
                            continue
                        n16 = nb // 16
                        for s0 in range(0, nb, MAXGIDX):
                            m = min(MAXGIDX, nb - s0)
                            nc.gpsimd.dma_gather(
                                gt[:, cstart + s0 // P:cstart + (s0 + m) // P, :],
                                table[b * brows:(b + 1) * brows, :],
                                its[:, off16 + s0 // 16:off16 + (s0 + m) // 16],
                                m, m, P, single_packet=False, queue_num=b)
                        off16 += n16
                de_ = depool.tile([P, nchmax, P], F16, tag="DE")
                nc.sync.dma_start(out=de_[:, :nch_g, :],
                                  in_=dexpd[:, chbase:chbase + nch_g, :])
                chbase += nch_g

                l3state = []
                for t in tiles_g:
                    tl = t - tiles_g[0]
                    runs = plan.g_bruns[gi][t]
                    cols = [c for (c0, c1) in runs for c in range(c0, c1)]
                    gself = gspool.tile([P, P], F16, tag="gs")
                    nc.sync.dma_start(out=gself[:],
                                      in_=ltable[t * P:(t + 1) * P, :])
                    if cols:
                        ncht = len(cols)
                        oh = ohpool.tile([P, ncht, P], F16, tag="oh")
                        l0 = 0
                        for (c0, c1) in runs:
                            n = c1 - c0
                            nc.vector.tensor_tensor(
                                out=oh[:, l0:l0 + n, :],
                                in0=de_[:, c0:c1, :],
                                in1=iot[:, tl, 0:n, :],
                                op=mybir.AluOpType.is_equal)
                            l0 += n
                    ps = pspool.tile([P, P], F32, tag="ps", space="PSUM")
                    for j, col in enumerate(cols):
                        nc.tensor.matmul(
                            ps[:], lhsT=gt[:, col, :], rhs=oh[:, j, :],
                            start=(j == 0), stop=False)
                    # self-loop: S^T += gself^T (identity matmul, local rows)
                    nc.tensor.matmul(ps[:], lhsT=gself[:], rhs=ids[:],
                                     start=(len(cols) == 0), stop=True)
                    st = stpool.tile([P, P], F16, tag="st")
                    nc.scalar.copy(out=st[:], in_=ps[:])
                    pt = ptpool.tile([P, do_], F32, tag="pt", space="PSUM")
                    if has_bias:
                        # bias via rank-1 update: pt += (1/dv)[dst] x b[f], so
                        # the dv (or dv^2) scale below also adds the +b term
                        nc.tensor.matmul(pt[:], lhsT=st[:], rhs=ws[:, :do_],
                                         start=True, stop=False)
                        nc.tensor.matmul(pt[:], lhsT=ivs[:, t * P:(t + 1) * P],
                                         rhs=bs[:, :do_], start=False,
                                         stop=True)
                    else:
                        nc.tensor.matmul(pt[:], lhsT=st[:], rhs=ws[:, :do_],
                                         start=True, stop=True)
                    dv = dvs[:, t:t + 1]
                    if isrelu:
                        # dv*relu(dv*raw + b) == relu(dv^2*(raw + b/dv));
                        # fused mult+max on DVE keeps the Act queue free for
                        # the st copies that gate the W matmuls
                        h2 = hpool.tile([P, do_], F16, tag="h2")
                        nc.vector.tensor_scalar(
                            out=h2[:], in0=pt[:],
                            scalar1=dv2s[:, t:t + 1], scalar2=0.0,
                            op0=mybir.AluOpType.mult,
                            op1=mybir.AluOpType.max)
                        nc.sync.dma_start(out=gout[t * P:(t + 1) * P, :],
                                          in_=h2[:])
                    else:
                        # log_softmax, batched over the group's tiles so the
                        # Exp/Ln activation tables each load once per group
                        # (not per tile) and the final DVE subtract never
                        # waits at the queue head
                        hs = hpool.tile([P, do_], F32, tag="hs")
                        nc.vector.tensor_scalar_mul(out=hs[:], in0=pt[:],
                                                    scalar1=dv)
                        l3state.append((t, dv, pt, hs))
                if l3state:
                    # no max-subtraction: |h3| stays far below fp32 exp range
                    sms = []
                    for t, dv, pt, hs in l3state:
                        es = hpool.tile([P, do_], F32, tag="es")
                        sm = hpool.tile([P, 1], F32, tag="sm")
                        nc.scalar.activation(
                            out=es[:], in_=pt[:],
                            func=mybir.ActivationFunctionType.Exp,
                            scale=dv, accum_out=sm[:])
                        sms.append(sm)
                    lss = []
                    for sm in sms:
                        ls = hpool.tile([P, 1], F32, tag="ls")
                        nc.scalar.activation(
                            out=ls[:], in_=sm[:],
                            func=mybir.ActivationFunctionType.Ln)
                        lss.append(ls)
                    for (t, dv, pt, hs), ls in zip(l3state, lss):
                        os_ = hpool.tile([P, do_], F32, tag="os")
                        nc.vector.tensor_single_scalar(
                            out=os_[:], in_=hs[:], scalar=ls[:],
                            op=mybir.AluOpType.subtract)
                        nc.sync.dma_start(out=out[t * P:(t + 1) * P, :],
                                          in_=os_[:])
            if li == 0:
                nc.gpsimd.collective_compute(
                    "AllGather", mybir.AluOpType.bypass, replica_groups=rg,
                    ins=[g1loc[:, :]], outs=[g1full[:, :]])
            elif li == 1:
                nc.gpsimd.collective_compute(
                    "AllGather", mybir.AluOpType.bypass, replica_groups=rg,
                    ins=[g2loc[:, :]], outs=[g2full[:, :]])

    nc.compile()
    return nc


def _make_in_maps(plan, W1, b1, W2, b2, W3, b3):
    dout = W3.shape[1]
    # iotr[p, tl, j, d] = tl*128 + d  (repeated iota windows per group tile)
    base = np.arange(P, dtype=np.float32)
    iotr = np.zeros((P, GROUP_TILES, plan.nmax, P), dtype=np.float16)
    for tl in range(GROUP_TILES):
        iotr[:, tl, :, :] = (tl * P + base)[None, None, :]
    common = {
        "w1": np.asarray(W1, np.float16), "w2": np.asarray(W2, np.float16),
        "w3": np.asarray(W3, np.float16),
        "b1r": np.asarray(b1, np.float16)[None, :],
        "b2r": np.asarray(b2, np.float16)[None, :],
        "b3r": np.asarray(b3, np.float16)[None, :],
        "iotr": iotr,
        "ident": np.eye(P, dtype=np.float16),
    }
    in_maps = []
    for c in range(plan.n_cores):
        gidx, dstl, dexp = plan.per_core_inputs[c]
        m = dict(common)
        m["xloc"] = plan.xloc[c]
        m["gt1"] = plan.gt1[c]
        m["gidx"] = gidx
        m["dexp"] = dexp
        m["dinvl"] = plan.dinvl[c]
        m["dinv2"] = plan.dinv2l[c]
        m["invd"] = plan.invdl[c]
        in_maps.append(m)
    return in_maps


def run_gcn(x, edge_index, W1, b1, W2, b2, W3, b3, n_cores=NCORES,
            trace=False):
    plan = _Plan(np.asarray(x, np.float32), edge_index, n_cores)
    dout = np.asarray(W3).shape[1]
    has_bias = any(np.any(np.asarray(b)) for b in (b1, b2, b3))
    nc = _build_program(plan, dout, has_bias)
    in_maps = _make_in_maps(plan, W1, b1, W2, b2, W3, b3)
    res = bass_utils.run_bass_kernel_spmd(
        nc, in_maps, core_ids=list(range(n_cores)), trace=trace)
    outs = []
    for c in range(n_cores):
        lo = c * plan.nloc
        hi = min(lo + plan.nloc, plan.N)
        outs.append(res.results[c]["out"][: hi - lo])
    full = np.concatenate(outs, axis=0)
    return full, res


def kernel(x, edge_index, W1, b1, W2, b2, W3, b3):
    out, _ = run_gcn(x, edge_index, W1, b1, W2, b2, W3, b3)
    return out
